# revision 24
# baseline (speedup 1.0000x reference)
"""Vertical-X two-stage Conv4D kernel: out[b] = W^T @ X[b] @ W.

Stage A uses the pair's X stacked VERTICALLY as the stationary operand:
    lhsT = Xpair [128 parts=(b2,ij), 64 cols=kl]    (K=128 -> FWL-eligible)
    rhs  = W1stack [128, 72] block-diag [[W,0],[0,W]]
    out  = U^T [64 parts=kl, 72 free=(b2,m)] per pair
Pairs alternate PSUM column halves (tile_position col 0/64) so a 14-pair
group packs into ONE [128, 504] PSUM bank: partition (parity*64+kl).

Stage B is one matmul per group with a parity-block-diagonal stationary:
    lhsT = W2blk [128 parts=(par,kl), 72 cols=(par,n)]
    rhs  = U^T group [128, 504]
    out  = [72 parts=(par,n), 504 free=(slot,b2,m)]

Input DMA lands on all 128 partitions (full SBUF write width); host packs
partition-major contiguous runs.  bf16 end-to-end, PSUM fp32.
"""

import numpy as np
import ml_dtypes

import concourse.bass as bass
import concourse.bacc as bacc
import concourse.mybir as mybir
from concourse.tile import TileContext
from concourse.bass_utils import run_bass_kernel_spmd

N_CORES = 8
B = 8192
B_C = B // N_CORES            # 1024 batches per core
PAIRS = B_C // 2              # 512 batch pairs per core
PAIRS_PER_GROUP = 14          # 7 slots x 72 = 504 fp32 <= one 2KB PSUM bank
GROUPS_PER_SUPER = 4
SUPER = GROUPS_PER_SUPER * PAIRS_PER_GROUP  # 56 pairs per DMA chunk
BF16 = mybir.dt.bfloat16
F32 = mybir.dt.float32


def build_weights(kern: np.ndarray) -> np.ndarray:
    """[128, 144] bf16: cols 0-71 = W1stack block-diag [[W,0],[0,W]]
    (stage-A moving operand); cols 72-143 = W2blk parity-block-diag
    (stage-B stationary)."""
    kern = np.asarray(kern, np.float32)
    W = np.zeros((64, 36), np.float32)
    for ip in range(6):
        for jp in range(6):
            m = ip * 6 + jp
            for a in range(3):
                for e in range(3):
                    W[(ip + a) * 8 + (jp + e), m] = kern[a, e]
    wall = np.zeros((128, 144), np.float32)
    wall[0:64, 0:36] = W       # W1stack
    wall[64:128, 36:72] = W
    wall[0:64, 72:108] = W     # W2blk (parity blocks)
    wall[64:128, 108:144] = W
    return wall.astype(ml_dtypes.bfloat16)


_PROGRAM_CACHE = {}


def build_program() -> bass.Bass:
    if "nc" in _PROGRAM_CACHE:
        return _PROGRAM_CACHE["nc"]

    nc = bacc.Bacc()
    x = nc.dram_tensor("x", [128, PAIRS * 64], BF16, kind="ExternalInput")
    w = nc.dram_tensor("w", [128, 144], BF16, kind="ExternalInput")
    o = nc.dram_tensor("o", [72, (PAIRS // 2) * 72], BF16, kind="ExternalOutput")

    with TileContext(nc) as tc:
        with (
            tc.tile_pool(name="wp", bufs=1) as wp,
            tc.tile_pool(name="xp", bufs=3) as xp,
            tc.tile_pool(name="up", bufs=3) as up,
            tc.tile_pool(name="pu", bufs=2, space="PSUM") as pu,
            tc.tile_pool(name="po", bufs=2, space="PSUM") as po,
            tc.tile_pool(name="op", bufs=3) as op,
        ):
            wt = wp.tile([128, 144], BF16)
            nc.sync.dma_start(out=wt[:, :], in_=w[:, :])

            off = 0
            ooff = 0
            left = PAIRS
            first = True
            sup = 0
            while left > 0:
                spairs = min(PAIRS_PER_GROUP if first else SUPER, left)
                first = False
                left -= spairs

                xg = xp.tile([128, SUPER * 64], BF16, tag="xg")
                half = (spairs // 2) * 64
                if half:
                    nc.sync.dma_start(
                        out=xg[:, :half], in_=x[:, off : off + half]
                    )
                nc.scalar.dma_start(
                    out=xg[:, half : spairs * 64],
                    in_=x[:, off + half : off + spairs * 64],
                )
                og = op.tile([72, (SUPER // 2) * 72], BF16, tag="og")

                done = 0
                while done < spairs:
                    npair = min(PAIRS_PER_GROUP, spairs - done)
                    nslot = npair // 2
                    nfree = nslot * 72

                    psu = pu.tile([128, (PAIRS_PER_GROUP // 2) * 72], F32, tag="psu")
                    for g in range(npair):
                        p = done + g
                        par, slot = g % 2, g // 2
                        nc.tensor.matmul(
                            psu[64 * par : 64 * par + 64, slot * 72 : slot * 72 + 72],
                            xg[:, p * 64 : p * 64 + 64],
                            wt[:, 0:72],
                            start=True,
                            stop=True,
                        )
                    ut = up.tile([128, (PAIRS_PER_GROUP // 2) * 72], BF16, tag="ut")
                    nc.scalar.copy(out=ut[:, :nfree], in_=psu[:, :nfree])

                    pso = po.tile([72, (PAIRS_PER_GROUP // 2) * 72], F32, tag="pso")
                    nc.tensor.matmul(
                        pso[:, :nfree],
                        wt[:, 72:144],
                        ut[:, :nfree],
                        start=True,
                        stop=True,
                    )
                    nc.vector.tensor_copy(
                        og[:, (done // 2) * 72 : (done // 2) * 72 + nfree],
                        pso[:, :nfree],
                    )
                    done += npair

                # Alternate the out-DMA between the two HWDGE queues so
                # neither queue carries input-half + all of the output.
                oq = nc.sync if sup % 2 else nc.scalar
                oq.dma_start(
                    out=o[:, ooff : ooff + (spairs // 2) * 72],
                    in_=og[:, : (spairs // 2) * 72],
                )

                off += spairs * 64
                ooff += (spairs // 2) * 72
                sup += 1

    nc.finalize()
    _PROGRAM_CACHE["nc"] = nc
    return nc


def pack_input(x_core: np.ndarray) -> np.ndarray:
    """[1024, 64, 64] f32 -> [128, PAIRS*64] bf16:
    A[b2*64+ij, p*64+kl] = x[2p+b2, ij, kl]."""
    a = x_core.reshape(PAIRS, 2, 64, 64).transpose(1, 2, 0, 3)
    return np.ascontiguousarray(a.reshape(128, PAIRS * 64)).astype(
        ml_dtypes.bfloat16
    )


def unpack_output(o_dev: np.ndarray) -> np.ndarray:
    """[72, (PAIRS//2)*72] bf16 -> [1024, 6,6,6,6] f32:
    o[par*36+n, pq*72 + b2*36 + m] = out[2*(2*pq+par)+b2, m, n]."""
    a = (
        o_dev.astype(np.float32)
        .reshape(2, 36, PAIRS // 2, 2, 36)
        .transpose(2, 0, 3, 4, 1)
    )
    return a.reshape(B_C, 6, 6, 6, 6)


def run(input_tensor: np.ndarray, kern: np.ndarray, **spmd_kwargs):
    input_tensor = np.ascontiguousarray(np.asarray(input_tensor, np.float32))
    wall = build_weights(kern)
    xs = input_tensor.reshape(N_CORES, B_C, 64, 64)
    in_maps = [{"x": pack_input(xs[c]), "w": wall} for c in range(N_CORES)]
    nc = build_program()
    res = run_bass_kernel_spmd(nc, in_maps, core_ids=list(range(N_CORES)), **spmd_kwargs)
    out = np.concatenate([unpack_output(r["o"]) for r in res.results], axis=0)
    return out, res


def kernel(input_tensor: np.ndarray, kernel: np.ndarray) -> np.ndarray:
    out, _ = run(input_tensor, kernel)
    return out


# revision 25
# speedup vs baseline: 1.1574x; 1.1574x over previous
"""Vertical-X two-stage Conv4D kernel: out[b] = W^T @ X[b] @ W.

Stage A uses the pair's X stacked VERTICALLY as the stationary operand:
    lhsT = Xpair [128 parts=(b2,ij), 64 cols=kl]    (K=128 -> FWL-eligible)
    rhs  = W1stack [128, 72] block-diag [[W,0],[0,W]]
    out  = U^T [64 parts=kl, 72 free=(b2,m)] per pair
Pairs alternate PSUM column halves (tile_position col 0/64) so a 14-pair
group packs into ONE [128, 504] PSUM bank: partition (parity*64+kl).

Stage B is one matmul per group with a parity-block-diagonal stationary:
    lhsT = W2blk [128 parts=(par,kl), 72 cols=(par,n)]
    rhs  = U^T group [128, 504]
    out  = [72 parts=(par,n), 504 free=(slot,b2,m)]

Input DMA lands on all 128 partitions (full SBUF write width); host packs
partition-major contiguous runs.  bf16 end-to-end, PSUM fp32.
"""

import numpy as np
import ml_dtypes

import concourse.bass as bass
import concourse.bacc as bacc
import concourse.mybir as mybir
from concourse.tile import TileContext
from concourse.bass_utils import run_bass_kernel_spmd

N_CORES = 8
B = 8192
B_C = B // N_CORES            # 1024 batches per core
PAIRS = B_C // 2              # 512 batch pairs per core
PAIRS_PER_GROUP = 14          # 7 slots x 72 = 504 fp32 <= one 2KB PSUM bank
GROUPS_PER_SUPER = 4
SUPER = GROUPS_PER_SUPER * PAIRS_PER_GROUP  # 56 pairs per DMA chunk
BF16 = mybir.dt.bfloat16
F32 = mybir.dt.float32


def build_weights(kern: np.ndarray) -> np.ndarray:
    """[128, 144] bf16: cols 0-71 = W1stack block-diag [[W,0],[0,W]]
    (stage-A moving operand); cols 72-143 = W2blk parity-block-diag
    (stage-B stationary)."""
    kern = np.asarray(kern, np.float32)
    W = np.zeros((64, 36), np.float32)
    for ip in range(6):
        for jp in range(6):
            m = ip * 6 + jp
            for a in range(3):
                for e in range(3):
                    W[(ip + a) * 8 + (jp + e), m] = kern[a, e]
    wall = np.zeros((128, 144), np.float32)
    wall[0:64, 0:36] = W       # W1stack
    wall[64:128, 36:72] = W
    wall[0:64, 72:108] = W     # W2blk (parity blocks)
    wall[64:128, 108:144] = W
    return wall.astype(ml_dtypes.bfloat16)


_PROGRAM_CACHE = {}


def build_program() -> bass.Bass:
    if "nc" in _PROGRAM_CACHE:
        return _PROGRAM_CACHE["nc"]

    nc = bacc.Bacc()
    x = nc.dram_tensor("x", [128, PAIRS * 64], BF16, kind="ExternalInput")
    w = nc.dram_tensor("w", [128, 144], BF16, kind="ExternalInput")
    o = nc.dram_tensor("o", [72, (PAIRS // 2) * 72], BF16, kind="ExternalOutput")

    with TileContext(nc) as tc:
        with (
            tc.tile_pool(name="wp", bufs=1) as wp,
            tc.tile_pool(name="xp", bufs=3) as xp,
            tc.tile_pool(name="up", bufs=3) as up,
            tc.tile_pool(name="pu", bufs=2, space="PSUM") as pu,
            tc.tile_pool(name="po", bufs=2, space="PSUM") as po,
            tc.tile_pool(name="op", bufs=3) as op,
        ):
            wt = wp.tile([128, 144], BF16)
            nc.sync.dma_start(out=wt[:, :], in_=w[:, :])

            off = 0
            ooff = 0
            left = PAIRS
            first = True
            while left > 0:
                spairs = min(PAIRS_PER_GROUP if first else SUPER, left)
                first = False
                left -= spairs

                xg = xp.tile([128, SUPER * 64], BF16, tag="xg")
                half = (spairs // 2) * 64
                if half:
                    nc.sync.dma_start(
                        out=xg[:, :half], in_=x[:, off : off + half]
                    )
                nc.scalar.dma_start(
                    out=xg[:, half : spairs * 64],
                    in_=x[:, off + half : off + spairs * 64],
                )
                og = op.tile([72, (SUPER // 2) * 72], BF16, tag="og")

                done = 0
                while done < spairs:
                    npair = min(PAIRS_PER_GROUP, spairs - done)
                    nslot = npair // 2
                    nfree = nslot * 72

                    psu = pu.tile([128, (PAIRS_PER_GROUP // 2) * 72], F32, tag="psu")
                    for g in range(npair):
                        p = done + g
                        par, slot = g % 2, g // 2
                        nc.tensor.matmul(
                            psu[64 * par : 64 * par + 64, slot * 72 : slot * 72 + 72],
                            xg[:, p * 64 : p * 64 + 64],
                            wt[:, 0:72],
                            start=True,
                            stop=True,
                        )
                    ut = up.tile([128, (PAIRS_PER_GROUP // 2) * 72], BF16, tag="ut")
                    nc.scalar.copy(out=ut[:, :nfree], in_=psu[:, :nfree])

                    pso = po.tile([72, (PAIRS_PER_GROUP // 2) * 72], F32, tag="pso")
                    nc.tensor.matmul(
                        pso[:, :nfree],
                        wt[:, 72:144],
                        ut[:, :nfree],
                        start=True,
                        stop=True,
                    )
                    nc.vector.tensor_copy(
                        og[:, (done // 2) * 72 : (done // 2) * 72 + nfree],
                        pso[:, :nfree],
                    )
                    done += npair

                nc.sync.dma_start(
                    out=o[:, ooff : ooff + (spairs // 2) * 72],
                    in_=og[:, : (spairs // 2) * 72],
                )

                off += spairs * 64
                ooff += (spairs // 2) * 72

    nc.finalize()
    _PROGRAM_CACHE["nc"] = nc
    return nc


def pack_input(x_core: np.ndarray) -> np.ndarray:
    """[1024, 64, 64] f32 -> [128, PAIRS*64] bf16:
    A[b2*64+ij, p*64+kl] = x[2p+b2, ij, kl]."""
    a = x_core.reshape(PAIRS, 2, 64, 64).transpose(1, 2, 0, 3)
    return np.ascontiguousarray(a.reshape(128, PAIRS * 64)).astype(
        ml_dtypes.bfloat16
    )


def unpack_output(o_dev: np.ndarray) -> np.ndarray:
    """[72, (PAIRS//2)*72] bf16 -> [1024, 6,6,6,6] f32:
    o[par*36+n, pq*72 + b2*36 + m] = out[2*(2*pq+par)+b2, m, n]."""
    a = (
        o_dev.astype(np.float32)
        .reshape(2, 36, PAIRS // 2, 2, 36)
        .transpose(2, 0, 3, 4, 1)
    )
    return a.reshape(B_C, 6, 6, 6, 6)


def run(input_tensor: np.ndarray, kern: np.ndarray, **spmd_kwargs):
    input_tensor = np.ascontiguousarray(np.asarray(input_tensor, np.float32))
    wall = build_weights(kern)
    xs = input_tensor.reshape(N_CORES, B_C, 64, 64)
    in_maps = [{"x": pack_input(xs[c]), "w": wall} for c in range(N_CORES)]
    nc = build_program()
    res = run_bass_kernel_spmd(nc, in_maps, core_ids=list(range(N_CORES)), **spmd_kwargs)
    out = np.concatenate([unpack_output(r["o"]) for r in res.results], axis=0)
    return out, res


def kernel(input_tensor: np.ndarray, kernel: np.ndarray) -> np.ndarray:
    out, _ = run(input_tensor, kernel)
    return out


# revision 26
# speedup vs baseline: 1.2044x; 1.0406x over previous
"""Vertical-X two-stage Conv4D kernel: out[b] = W^T @ X[b] @ W.

Stage A uses the pair's X stacked VERTICALLY as the stationary operand:
    lhsT = Xpair [128 parts=(b2,ij), 64 cols=kl]    (K=128 -> FWL-eligible)
    rhs  = W1stack [128, 72] block-diag [[W,0],[0,W]]
    out  = U^T [64 parts=kl, 72 free=(b2,m)] per pair
Pairs alternate PSUM column halves (tile_position col 0/64) so a 14-pair
group packs into ONE [128, 504] PSUM bank: partition (parity*64+kl).

Stage B is one matmul per group with a parity-block-diagonal stationary:
    lhsT = W2blk [128 parts=(par,kl), 72 cols=(par,n)]
    rhs  = U^T group [128, 504]
    out  = [72 parts=(par,n), 504 free=(slot,b2,m)]

Input DMA lands on all 128 partitions (full SBUF write width); host packs
partition-major contiguous runs.  bf16 end-to-end, PSUM fp32.
"""

import numpy as np
import ml_dtypes

import concourse.bass as bass
import concourse.bacc as bacc
import concourse.mybir as mybir
from concourse.tile import TileContext
from concourse.bass_utils import run_bass_kernel_spmd

N_CORES = 8
B = 8192
B_C = B // N_CORES            # 1024 batches per core
PAIRS = B_C // 2              # 512 batch pairs per core
PAIRS_PER_GROUP = 14          # 7 slots x 72 = 504 fp32 <= one 2KB PSUM bank
GROUPS_PER_SUPER = 4
SUPER = GROUPS_PER_SUPER * PAIRS_PER_GROUP  # 56 pairs per DMA chunk
BF16 = mybir.dt.bfloat16
F32 = mybir.dt.float32


def build_weights(kern: np.ndarray) -> np.ndarray:
    """[128, 144] bf16: cols 0-71 = W1stack block-diag [[W,0],[0,W]]
    (stage-A moving operand); cols 72-143 = W2blk parity-block-diag
    (stage-B stationary)."""
    kern = np.asarray(kern, np.float32)
    W = np.zeros((64, 36), np.float32)
    for ip in range(6):
        for jp in range(6):
            m = ip * 6 + jp
            for a in range(3):
                for e in range(3):
                    W[(ip + a) * 8 + (jp + e), m] = kern[a, e]
    wall = np.zeros((128, 144), np.float32)
    wall[0:64, 0:36] = W       # W1stack
    wall[64:128, 36:72] = W
    wall[0:64, 72:108] = W     # W2blk (parity blocks)
    wall[64:128, 108:144] = W
    return wall.astype(ml_dtypes.bfloat16)


_PROGRAM_CACHE = {}


def build_program() -> bass.Bass:
    if "nc" in _PROGRAM_CACHE:
        return _PROGRAM_CACHE["nc"]

    nc = bacc.Bacc()
    x = nc.dram_tensor("x", [128, PAIRS * 64], BF16, kind="ExternalInput")
    w = nc.dram_tensor("w", [128, 144], BF16, kind="ExternalInput")
    o = nc.dram_tensor("o", [72, (PAIRS // 2) * 72], BF16, kind="ExternalOutput")

    with TileContext(nc) as tc:
        with (
            tc.tile_pool(name="wp", bufs=1) as wp,
            tc.tile_pool(name="xp", bufs=3) as xp,
            tc.tile_pool(name="up", bufs=3) as up,
            tc.tile_pool(name="pu", bufs=2, space="PSUM") as pu,
            tc.tile_pool(name="po", bufs=2, space="PSUM") as po,
            tc.tile_pool(name="op", bufs=3) as op,
        ):
            wt = wp.tile([128, 144], BF16)
            nc.sync.dma_start(out=wt[:, :], in_=w[:, :])

            off = 0
            ooff = 0
            left = PAIRS
            first = True
            while left > 0:
                spairs = min(PAIRS_PER_GROUP if first else SUPER, left)
                first = False
                left -= spairs

                xg = xp.tile([128, SUPER * 64], BF16, tag="xg")
                # One DMA per 14-pair group, alternating the two HWDGE
                # queues: each group's matmuls wait only on their own
                # group's chunk, not half the supergroup.
                gstart, qi = 0, 0
                while gstart < spairs:
                    gp = min(PAIRS_PER_GROUP, spairs - gstart)
                    q = nc.sync if qi % 2 == 0 else nc.scalar
                    q.dma_start(
                        out=xg[:, gstart * 64 : (gstart + gp) * 64],
                        in_=x[:, off + gstart * 64 : off + (gstart + gp) * 64],
                    )
                    gstart += gp
                    qi += 1
                og = op.tile([72, (SUPER // 2) * 72], BF16, tag="og")

                done = 0
                while done < spairs:
                    npair = min(PAIRS_PER_GROUP, spairs - done)
                    nslot = npair // 2
                    nfree = nslot * 72

                    psu = pu.tile([128, (PAIRS_PER_GROUP // 2) * 72], F32, tag="psu")
                    for g in range(npair):
                        p = done + g
                        par, slot = g % 2, g // 2
                        nc.tensor.matmul(
                            psu[64 * par : 64 * par + 64, slot * 72 : slot * 72 + 72],
                            xg[:, p * 64 : p * 64 + 64],
                            wt[:, 0:72],
                            start=True,
                            stop=True,
                        )
                    ut = up.tile([128, (PAIRS_PER_GROUP // 2) * 72], BF16, tag="ut")
                    nc.scalar.copy(out=ut[:, :nfree], in_=psu[:, :nfree])

                    pso = po.tile([72, (PAIRS_PER_GROUP // 2) * 72], F32, tag="pso")
                    nc.tensor.matmul(
                        pso[:, :nfree],
                        wt[:, 72:144],
                        ut[:, :nfree],
                        start=True,
                        stop=True,
                    )
                    nc.vector.tensor_copy(
                        og[:, (done // 2) * 72 : (done // 2) * 72 + nfree],
                        pso[:, :nfree],
                    )
                    done += npair

                nc.sync.dma_start(
                    out=o[:, ooff : ooff + (spairs // 2) * 72],
                    in_=og[:, : (spairs // 2) * 72],
                )

                off += spairs * 64
                ooff += (spairs // 2) * 72

    nc.finalize()
    _PROGRAM_CACHE["nc"] = nc
    return nc


def pack_input(x_core: np.ndarray) -> np.ndarray:
    """[1024, 64, 64] f32 -> [128, PAIRS*64] bf16:
    A[b2*64+ij, p*64+kl] = x[2p+b2, ij, kl]."""
    a = x_core.reshape(PAIRS, 2, 64, 64).transpose(1, 2, 0, 3)
    return np.ascontiguousarray(a.reshape(128, PAIRS * 64)).astype(
        ml_dtypes.bfloat16
    )


def unpack_output(o_dev: np.ndarray) -> np.ndarray:
    """[72, (PAIRS//2)*72] bf16 -> [1024, 6,6,6,6] f32:
    o[par*36+n, pq*72 + b2*36 + m] = out[2*(2*pq+par)+b2, m, n]."""
    a = (
        o_dev.astype(np.float32)
        .reshape(2, 36, PAIRS // 2, 2, 36)
        .transpose(2, 0, 3, 4, 1)
    )
    return a.reshape(B_C, 6, 6, 6, 6)


def run(input_tensor: np.ndarray, kern: np.ndarray, **spmd_kwargs):
    input_tensor = np.ascontiguousarray(np.asarray(input_tensor, np.float32))
    wall = build_weights(kern)
    xs = input_tensor.reshape(N_CORES, B_C, 64, 64)
    in_maps = [{"x": pack_input(xs[c]), "w": wall} for c in range(N_CORES)]
    nc = build_program()
    res = run_bass_kernel_spmd(nc, in_maps, core_ids=list(range(N_CORES)), **spmd_kwargs)
    out = np.concatenate([unpack_output(r["o"]) for r in res.results], axis=0)
    return out, res


def kernel(input_tensor: np.ndarray, kernel: np.ndarray) -> np.ndarray:
    out, _ = run(input_tensor, kernel)
    return out
